# revision 1
# baseline (speedup 1.0000x reference)
"""Embedding lookup on 8 Trainium2 NeuronCores.

out[b, s, :] = W[:, input[b, s]]   (W: [d_model, vocab])

Strategy: data-parallel over the 16384 tokens (2048 per core); the gather
table is replicated. The host transposes W once to W_T [vocab, d_model] so
each embedding row is a contiguous 4KB run, then each core performs
indirect-DMA row gathers (128 rows x 4KB per op) and stores contiguous
[128, 1024] slabs to its output shard.
"""
import sys

sys.path.insert(0, "/opt/trn_rl_repo")

import numpy as np

import concourse.bass as bass
import concourse.tile as tile
from concourse import bacc, mybir
from concourse.bass_utils import run_bass_kernel_spmd

VOCAB = 50257
D_MODEL = 1024
BATCH = 4
SEQ = 4096
N_CORES = 8
P = 128

TOKENS = BATCH * SEQ              # 16384
T_CORE = TOKENS // N_CORES        # 2048 tokens per core
NT = T_CORE // P                  # 16 gather tiles of 128 tokens

_compiled = None


def _build():
    nc = bacc.Bacc("TRN2", target_bir_lowering=False, debug=False,
                   num_devices=N_CORES)
    table = nc.dram_tensor("table", [VOCAB, D_MODEL], mybir.dt.float32,
                           kind="ExternalInput").ap()
    idx = nc.dram_tensor("idx", [T_CORE], mybir.dt.int32,
                         kind="ExternalInput").ap()
    out = nc.dram_tensor("out", [T_CORE, D_MODEL], mybir.dt.float32,
                         kind="ExternalOutput").ap()

    with tile.TileContext(nc) as tc:
        with tc.tile_pool(name="idxp", bufs=1) as idxp, \
             tc.tile_pool(name="rows", bufs=4) as rows:
            idx_tile = idxp.tile([P, NT], mybir.dt.int32)
            # idx_tile[p, t] = idx[t*128 + p]
            nc.sync.dma_start(idx_tile[:, :], idx.rearrange("(t p) -> p t", p=P))
            for t in range(NT):
                g = rows.tile([P, D_MODEL], mybir.dt.float32)
                nc.gpsimd.indirect_dma_start(
                    out=g[:, :],
                    out_offset=None,
                    in_=table[:, :],
                    in_offset=bass.IndirectOffsetOnAxis(
                        ap=idx_tile[:, t:t + 1], axis=0),
                )
                nc.sync.dma_start(out[t * P:(t + 1) * P, :], g[:, :])
    nc.compile()
    return nc


def kernel(input: np.ndarray, W: np.ndarray) -> np.ndarray:
    global _compiled
    assert input.shape == (BATCH, SEQ) and W.shape == (D_MODEL, VOCAB)
    if _compiled is None:
        _compiled = _build()
    nc = _compiled

    table_np = np.ascontiguousarray(np.asarray(W, dtype=np.float32).T)
    idx_flat = np.ascontiguousarray(
        np.asarray(input, dtype=np.int32).reshape(TOKENS))

    in_maps = [
        {"table": table_np, "idx": idx_flat[k * T_CORE:(k + 1) * T_CORE]}
        for k in range(N_CORES)
    ]
    res = run_bass_kernel_spmd(nc, in_maps, core_ids=list(range(N_CORES)))
    out = np.concatenate([res.results[k]["out"] for k in range(N_CORES)], axis=0)
    return out.reshape(BATCH, SEQ, D_MODEL)


# revision 2
# speedup vs baseline: 1.1332x; 1.1332x over previous
"""Embedding lookup on 8 Trainium2 NeuronCores.

out[b, s, :] = W[:, input[b, s]]   (W: [d_model, vocab])

Strategy: data-parallel over the 16384 tokens (2048 per core); the gather
table is replicated. The host transposes W once to W_T [vocab, d_model] so
each embedding row is a contiguous 4KB run, then each core runs a raw-Bass
pipeline: 16 indirect-DMA row gathers (128 rows x 4KB per op, SWDGE) each
paired with a HWDGE store of the landed [128, 1024] slab to the core's
output shard. Gathers and stores overlap; per-op semaphores carry the
exact gather->store dependency (a single cumulative semaphore would be
racy: 16 increments can come from engines running ahead on later ops).
"""
import sys

sys.path.insert(0, "/opt/trn_rl_repo")

import contextlib

import numpy as np

import concourse.bass as bass
from concourse import mybir
from concourse.bass_utils import run_bass_kernel_spmd

VOCAB = 50257
D_MODEL = 1024
BATCH = 4
SEQ = 4096
N_CORES = 8
P = 128

TOKENS = BATCH * SEQ              # 16384
T_CORE = TOKENS // N_CORES        # 2048 tokens per core
NT = T_CORE // P                  # 16 gather ops of 128 rows

_compiled = None


def _build():
    nc = bass.Bass("TRN2", debug=False, num_devices=N_CORES)
    table = nc.dram_tensor("table", [VOCAB, D_MODEL], mybir.dt.float32,
                           kind="ExternalInput")
    idx = nc.dram_tensor("idx", [T_CORE], mybir.dt.int32, kind="ExternalInput")
    out = nc.dram_tensor("out", [T_CORE, D_MODEL], mybir.dt.float32,
                         kind="ExternalOutput")

    with contextlib.ExitStack() as st:
        idx_tile = st.enter_context(nc.sbuf_tensor([P, NT], mybir.dt.int32))
        gbuf = st.enter_context(
            nc.sbuf_tensor([P, NT * D_MODEL], mybir.dt.float32))
        idx_sem = st.enter_context(nc.semaphore("idx_sem"))
        g_sems = [st.enter_context(nc.semaphore(f"g{t}")) for t in range(NT)]
        s_sem = st.enter_context(nc.semaphore("s_sem"))
        block = st.enter_context(nc.Block())

        # out viewed as [P, NT, D]: row p*NT + t holds token p*NT + t, which
        # matches idx_tile[p, t] = idx[p*NT + t] below.
        out_v = out.ap().rearrange("(p t) d -> p t d", t=NT)

        @block.sync
        def _(sync):
            sync.dma_start(idx_tile[:, :],
                           idx.ap().rearrange("(p t) -> p t", p=P)
                           ).then_inc(idx_sem, 16)
            for t in range(NT):
                sync.wait_ge(g_sems[t], 16)
                sync.dma_start(out_v[:, t, :],
                               gbuf[:, t * D_MODEL:(t + 1) * D_MODEL]
                               ).then_inc(s_sem, 16)
            # 16 stores x 16 engine-increments each: all store data landed.
            sync.wait_ge(s_sem, 16 * NT)

        @block.gpsimd
        def _(gpsimd):
            gpsimd.wait_ge(idx_sem, 16)
            for t in range(NT):
                gpsimd.indirect_dma_start(
                    out=gbuf[:, t * D_MODEL:(t + 1) * D_MODEL],
                    out_offset=None,
                    in_=table.ap(),
                    in_offset=bass.IndirectOffsetOnAxis(
                        ap=idx_tile[:, t:t + 1], axis=0),
                ).then_inc(g_sems[t], 16)

    return nc


def kernel(input: np.ndarray, W: np.ndarray) -> np.ndarray:
    global _compiled
    assert input.shape == (BATCH, SEQ) and W.shape == (D_MODEL, VOCAB)
    if _compiled is None:
        _compiled = _build()
    nc = _compiled

    table_np = np.ascontiguousarray(np.asarray(W, dtype=np.float32).T)
    idx_flat = np.ascontiguousarray(
        np.asarray(input, dtype=np.int32).reshape(TOKENS))

    in_maps = [
        {"table": table_np, "idx": idx_flat[k * T_CORE:(k + 1) * T_CORE]}
        for k in range(N_CORES)
    ]
    res = run_bass_kernel_spmd(nc, in_maps, core_ids=list(range(N_CORES)))
    out = np.concatenate([res.results[k]["out"] for k in range(N_CORES)], axis=0)
    return out.reshape(BATCH, SEQ, D_MODEL)
